# revision 1
# baseline (speedup 1.0000x reference)
"""Position-attention kernel for Trainium2 (8 NeuronCores, SPMD data-parallel).

Math (per batch b):
    q = X Wq ; k = X Wk ; v = X Wv          (X = x[b] reshaped [N, C], N=4096, C=128)
    energy[i, j] = k_i . q_j
    attn = softmax(energy, axis=-1)
    out = gamma * (attn @ v) + X

Kernel restructuring:
    energy = X A X^T with A = Wq Wk^T, computed transposed as
    eT[j, i] = sum_c xT[c, j] * w[c, i]  where  w = A @ X_i^T  (tiny prep matmul).
    eT lands in PSUM with j on partitions and is exp'd (softmax is shift/scale
    invariant) by the scalar engine directly into SBUF as bf16 -> already in
    the right layout to be the stationary operand of the attn@v matmul (no
    transposes anywhere). A ones-column appended to v gives the softmax
    denominator for free.

Sharding: 8 cores = (4 batches) x (2 halves of the 4096 output rows).
"""

import numpy as np

B, Dd, Hh, Ww, C = 4, 16, 16, 16, 128
N = Dd * Hh * Ww            # 4096 sequence positions (j)
NCORES = 8
NI = (B * N) // NCORES      # 2048 output rows per core (i)
NJB = N // 128              # 32 j-blocks
G = 6                       # j-blocks per exp group (PSUM: 2*3 + 2 banks = 8)
IC = 256                    # i-chunk (2 accumulator tiles of 128 rows)
NICH = NI // IC             # 8 i-chunks
NIT = IC // 128             # 2 i-tiles per chunk
SHIFT = 32.0                # softmax shift (cancels exactly in normalization)

_NC_CACHE = {}


def _build_nc():
    from contextlib import ExitStack

    import concourse.bacc as bacc
    import concourse.bass as bass
    import concourse.mybir as mybir
    import concourse.tile as tile

    dt = mybir.dt
    nc = bacc.Bacc(target_bir_lowering=False)

    xT_d = nc.declare_dram_parameter("xT", [128, N], dt.float16, isOutput=False)
    xres_d = nc.declare_dram_parameter(
        "xres", [NI // 128, 128, 128], dt.float32, isOutput=False
    )
    # weights packed [A^T | Wv] along columns to halve DMA count
    aw_d = nc.declare_dram_parameter("aw", [128, 256], dt.float16, isOutput=False)
    gam_d = nc.declare_dram_parameter("gam", [1, 1], dt.float32, isOutput=False)
    out_d = nc.declare_dram_parameter(
        "out", [NI // 128, 128, 128], dt.float32, isOutput=True
    )

    NCH = N // 512   # 8 column chunks of xT
    NWC = NI // 512  # 4 column chunks of xTi / w
    with tile.TileContext(nc) as tc, ExitStack() as ctx:
        persist = ctx.enter_context(tc.tile_pool(name="persist", bufs=1))

        # warm up the exp table load while DMAs run
        dummy = persist.tile([1, 1], dt.float32)
        nc.vector.memset(dummy[:], 0.0)
        nc.scalar.activation(
            out=dummy[:], in_=dummy[:], func=mybir.ActivationFunctionType.Exp
        )
        # zeroed operand for PE-warmup matmuls
        warm = persist.tile([128, 128], dt.float16)
        nc.vector.memset(warm[:], 0.0)

        # DMA order mirrors the critical path: tiny queue-warmers first, then
        # aw -> xti[0:256] -> xt0/xt1 on the sync queue feed the first energy
        # group; the scalar queue carries the rest.
        qw = persist.tile([1, 4], dt.float16)
        nc.sync.dma_start(out=qw[0:1, 0:2], in_=aw_d[0:1, 0:2])
        nc.scalar.dma_start(out=qw[0:1, 2:4], in_=aw_d[0:1, 2:4])
        aw = persist.tile([128, 2, 128], dt.float16)
        xt_ch = [
            persist.tile([128, 512], dt.float16, name=f"xt{jc}") for jc in range(NCH)
        ]
        v_ch = [
            persist.tile([128, 4, 132], dt.bfloat16, name=f"v{jc}")
            for jc in range(NCH)
        ]

        def ld_xt(eng, jc):
            eng.dma_start(out=xt_ch[jc][:], in_=xT_d[:, jc * 512 : (jc + 1) * 512])

        # x is j-rotated on the host so each core's own i-rows are chunks 0-3;
        # interleave the two HW queues so the first-group inputs land first
        nc.sync.dma_start(out=aw[:], in_=aw_d[:, :])
        nc.scalar.dma_start(out=xt_ch[0][:, 0:256], in_=xT_d[:, 0:256])
        nc.sync.dma_start(out=xt_ch[0][:, 256:512], in_=xT_d[:, 256:512])
        for jc, eng in [(1, nc.scalar), (2, nc.scalar), (3, nc.sync),
                        (4, nc.scalar), (5, nc.sync), (6, nc.scalar), (7, nc.sync)]:
            ld_xt(eng, jc)
        gam = persist.tile([128, 1], dt.float32)
        gam_ap = gam_d[:, :]
        nc.gpsimd.dma_start(
            out=gam[:],
            in_=bass.AP(
                tensor=gam_ap.tensor, offset=gam_ap.offset, ap=[[0, 128], [1, 1]]
            ),
        )
        shiftb = persist.tile([128, 1], dt.float32)
        nc.vector.memset(shiftb[:], -SHIFT)
        for jc in range(NCH):
            nc.vector.memset(v_ch[jc][:, :, 128:129], 1.0)

        at_s = aw[:, 0, :]
        wv_s = aw[:, 1, :]
        w_ch = [persist.tile([128, 512], dt.float16, name=f"w{k}") for k in range(NWC)]

        epool = ctx.enter_context(tc.tile_pool(name="epsum", bufs=2, space="PSUM"))
        opool = ctx.enter_context(tc.tile_pool(name="opsum", bufs=1, space="PSUM"))
        ptpool = ctx.enter_context(tc.tile_pool(name="ptp", bufs=8))
        spool = ctx.enter_context(tc.tile_pool(name="small", bufs=8))
        osb_pool = ctx.enter_context(tc.tile_pool(name="osb", bufs=3))
        xrpool = ctx.enter_context(tc.tile_pool(name="xrp", bufs=3))
        outpool = ctx.enter_context(tc.tile_pool(name="outp", bufs=3))

        def emit_vprep(jc, tag):
            t = opool.tile([128, 4, 128], dt.float32, tag=tag, name=f"vp{jc}")
            for k in range(4):
                nc.tensor.matmul(
                    t[:, k, :],
                    xt_ch[jc][:, k * 128 : (k + 1) * 128],
                    wv_s,
                    start=True,
                    stop=True,
                )
            nc.vector.tensor_copy(out=v_ch[jc][:, :, 0:128], in_=t[:])

        def emit_wprep(k, tag, half=None):
            if half is None:
                t = opool.tile([128, 512], dt.float32, tag=tag, name=f"wp{k}")
                nc.tensor.matmul(t[:], at_s, xt_ch[k][:], start=True, stop=True)
                nc.vector.tensor_copy(out=w_ch[k][:], in_=t[:])
            else:
                sl = slice(half * 256, half * 256 + 256)
                t = opool.tile([128, 256], dt.float32, tag=tag, name=f"wp{k}_{half}")
                nc.tensor.matmul(t[:], at_s, xt_ch[k][:, sl], start=True, stop=True)
                nc.vector.tensor_copy(out=w_ch[k][:, sl], in_=t[:])

        # ---- prep: PE warmup + the single critical w half (i-cols 0:256) ----
        wt = opool.tile([128, 512], dt.float32, tag="oa0", name="warmp")
        for r in range(8):
            nc.tensor.matmul(
                wt[:, r * 64 : (r + 1) * 64],
                warm[:],
                warm[:, 0:64],
                start=True,
                stop=True,
            )
        emit_wprep(0, "oa0", half=0)

        # ---- main loop ----
        ngroups = (NJB + G - 1) // G
        # ic 0 starts with a chunk-0-only group so the first exp waits on the
        # minimum set of DMAs; later ics use the regular split
        GROUPS0 = [(0, 4), (4, 6), (10, 6), (16, 6), (22, 6), (28, 4)]
        GROUPSN = [(0, 6), (6, 6), (12, 6), (18, 6), (24, 6), (30, 2)]

        def groups_of(icn):
            return GROUPS0 if icn == 0 else GROUPSN

        # all remaining prep (v chunks, later w chunks) is injected into ic 0's
        # groups through the oa-tag psum banks, once its xT chunk has landed;
        # attn@v lags correspondingly but catches up (pt pool gives slack)
        prep_at = {
            0: (("v", 0, "oa0"),),
            1: (("v", 1, "oa1"), ("v", 2, "oa0")),
            2: (("v", 3, "oa1"), ("v", 4, "oa0")),
            3: (("v", 5, "oa1"), ("v", 6, "oa0")),
            4: (("v", 7, "oa1"), ("wh", 0, "oa0")),
            5: (("w", 1, "oa1"), ("w", 2, "oa0"), ("w", 3, "oa1")),
        }

        def emit_energy(icn, gi):
            jb0, gsz = groups_of(icn)[gi]
            et = epool.tile([128, G, IC], dt.float32, tag="et", name=f"et{icn}_{gi}")
            wsl = w_ch[icn // 2][:, (icn % 2) * IC : (icn % 2 + 1) * IC]
            for g in range(gsz):
                jb = jb0 + g
                nc.tensor.matmul(
                    et[:, g, :],
                    xt_ch[jb // 4][:, (jb % 4) * 128 : (jb % 4 + 1) * 128],
                    wsl,
                    start=True,
                    stop=True,
                )
            return et

        def emit_exp(icn, gi):
            gsz = groups_of(icn)[gi][1]
            et = ets.pop((icn, gi))
            pt = ptpool.tile(
                [128, G, IC], dt.bfloat16, tag="pt", name=f"pt{icn}_{gi}"
            )
            nc.scalar.activation(
                out=pt[:, :gsz, :],
                in_=et[:, :gsz, :],
                func=mybir.ActivationFunctionType.Exp,
                bias=shiftb[:],
            )
            return pt

        def emit_attnv(icn, gi, pt):
            jb0, gsz = groups_of(icn)[gi]
            oa = oa_by_ic[icn]
            for g in range(gsz):
                jb = jb0 + g
                for it in range(NIT):
                    nc.tensor.matmul(
                        oa[it][:],
                        pt[:, g, it * 128 : (it + 1) * 128],
                        v_ch[jb // 4][:, jb % 4, 0:129],
                        start=(jb == 0),
                        stop=(jb == NJB - 1),
                    )

        def alloc_oa(icn):
            oa_by_ic[icn] = [
                opool.tile([128, 129], dt.float32, tag=f"oa{k}", name=f"oa{k}_{icn}")
                for k in range(NIT)
            ]

        def emit_blend(icn):
            oa = oa_by_ic[icn]
            for it in range(NIT):
                ti = icn * NIT + it
                # single fast PSUM read frees the accumulator bank quickly
                osb = osb_pool.tile([128, 129], dt.float32, tag="osb", name=f"osb{ti}")
                nc.vector.tensor_copy(out=osb[:], in_=oa[it][:])
                rs = spool.tile([128, 1], dt.float32, tag="rs", name=f"rs{ti}")
                nc.vector.reciprocal(rs[:], osb[:, 128:129])
                nc.vector.tensor_scalar(
                    out=rs[:],
                    in0=rs[:],
                    scalar1=gam[:],
                    scalar2=None,
                    op0=mybir.AluOpType.mult,
                )
                xr = xrpool.tile([128, 128], dt.float32, tag="xr", name=f"xr{ti}")
                nc.sync.dma_start(out=xr[:], in_=xres_d[ti])
                ot = outpool.tile([128, 128], dt.float32, tag="ot", name=f"ot{ti}")
                nc.vector.tensor_scalar(
                    out=ot[:],
                    in0=osb[:, 0:128],
                    scalar1=rs[:],
                    scalar2=None,
                    op0=mybir.AluOpType.mult,
                )
                nc.vector.tensor_tensor(
                    out=ot[:], in0=ot[:], in1=xr[:], op=mybir.AluOpType.add
                )
                nc.sync.dma_start(out=out_d[ti], in_=ot[:])

        ets = {}
        oa_by_ic = {}
        pts = {}

        # ---- i-chunk 0: energies/exps run at full cadence while all prep
        # (v chunks, remaining w) flows through the oa psum banks; the attn@v
        # matmuls for ic 0 are deferred until the oa banks are free ----
        ets[(0, 0)] = emit_energy(0, 0)
        for gi in range(ngroups):
            if gi + 1 < ngroups:
                ets[(0, gi + 1)] = emit_energy(0, gi + 1)
            pts[gi] = emit_exp(0, gi)
            for kind, idx, tag in prep_at.get(gi, ()):
                if kind == "v":
                    emit_vprep(idx, tag)
                elif kind == "w":
                    emit_wprep(idx, tag)
                else:
                    emit_wprep(idx, tag, half=1)
        ets[(1, 0)] = emit_energy(1, 0)
        alloc_oa(0)
        for gi in range(ngroups):
            emit_attnv(0, gi, pts.pop(gi))
            # keep feeding the energy pipeline through the deferred burst
            nxt = (1, gi + 1) if gi + 1 < ngroups else (2, 0)
            ets[nxt] = emit_energy(*nxt)
        emit_blend(0)

        # ---- i-chunks 1..: flat schedule with two-group PE lookahead ACROSS
        # chunk boundaries, so the scalar engine never waits on energy ----
        flat = [(icn, gi) for icn in range(1, NICH) for gi in range(ngroups)]
        for fk, (icn, gi) in enumerate(flat):
            for ahead in (1, 2, 3):
                if fk + ahead < len(flat) and flat[fk + ahead] not in ets:
                    nicn, ngi = flat[fk + ahead]
                    ets[flat[fk + ahead]] = emit_energy(nicn, ngi)
            pt = emit_exp(icn, gi)
            if gi == 0:
                alloc_oa(icn)
            emit_attnv(icn, gi, pt)
            if gi == ngroups - 1:
                emit_blend(icn)

    nc.finalize()
    return nc


def get_nc():
    if "nc" not in _NC_CACHE:
        _NC_CACHE["nc"] = _build_nc()
    return _NC_CACHE["nc"]


def make_in_maps(x, Wq, Wk, Wv, gamma):
    x = np.asarray(x, dtype=np.float32)
    Wq = np.asarray(Wq, dtype=np.float32)
    Wk = np.asarray(Wk, dtype=np.float32)
    Wv = np.asarray(Wv, dtype=np.float32)
    gamma = np.asarray(gamma, dtype=np.float32)

    xf = x.reshape(B, N, C)
    A = Wq @ Wk.T
    aw = np.ascontiguousarray(
        np.concatenate([A.T.astype(np.float16), Wv.astype(np.float16)], axis=1)
    )  # [128, 256] = [A^T | Wv]
    gam = gamma.reshape(1, 1)

    in_maps = []
    for c in range(NCORES):
        b, ih = c // 2, c % 2
        xT = xf[b].T.astype(np.float16)  # [128, 4096]
        # rotate the j-order so this core's own i-rows are columns 0:NI
        # (softmax sums over j, so any j-order works as long as v matches)
        xTr = np.ascontiguousarray(np.roll(xT, -ih * NI, axis=1))
        sl = slice(ih * NI, (ih + 1) * NI)
        in_maps.append(
            {
                "xT": xTr,
                "xres": np.ascontiguousarray(
                    xf[b][sl].reshape(NI // 128, 128, 128)
                ),
                "aw": aw,
                "gam": gam,
            }
        )
    return in_maps


def assemble_out(results):
    outs = [np.asarray(results[c]["out"]).reshape(NI, C) for c in range(NCORES)]
    full = np.stack(
        [np.concatenate([outs[2 * b], outs[2 * b + 1]], axis=0) for b in range(B)]
    )
    return full.reshape(B, Dd, Hh, Ww, C).astype(np.float32)


def kernel(x, Wq, Wk, Wv, gamma):
    from concourse.bass_utils import run_bass_kernel_spmd

    nc = get_nc()
    in_maps = make_in_maps(x, Wq, Wk, Wv, gamma)
    res = run_bass_kernel_spmd(nc, in_maps, core_ids=list(range(NCORES)))
    return assemble_out(res.results)



# revision 4
# speedup vs baseline: 1.1620x; 1.1620x over previous
"""Position-attention kernel for Trainium2 (8 NeuronCores, SPMD data-parallel).

Math (per batch b):
    q = X Wq ; k = X Wk ; v = X Wv          (X = x[b] reshaped [N, C], N=4096)
    energy[i, j] = k_i . q_j ;  attn = softmax(energy, -1)
    out = gamma * (attn @ v) + X

v2 design (exp-bound pipeline):
  - Host precomputes qT/kT (rank-16 factors of the energy), gamma-folded V
    with a ones-column (softmax denominator for free), and the fp32 residual.
  - Energy eT[j,i] = qT^T kT via 32x32 tile-packed matmuls (tile_position):
    contraction is 17 wide (16 channels + a shift row baking in the -SHIFT
    softmax bias), so a 12-MM pack covers [384 j, 512 i] at ~4x PE efficiency.
  - exp on the scalar engine for most j-groups (FD=1536 activations); selected
    groups use a 1-op DVE Schraudolph exp: uint16(e*C1 + C2) bit-cast to bf16
    (saturating convert clamps the e^-90 underflow tail to +0).
  - attn@v: pt stationary [j, i-128], moving [v|1] (129 cols); one PSUM
    accumulation pass per 128-row output slice, two banks round-robin.
  - blend: reciprocal + scale on DVE, residual add on GPSIMD, out DMA on sync.

Sharding: 8 cores = (4 batches) x (2 halves of the 4096 output rows).
"""

import numpy as np

B, Dd, Hh, Ww, C = 4, 16, 16, 16, 128
N = Dd * Hh * Ww            # 4096 sequence positions (j)
NCORES = 8
NI = (B * N) // NCORES      # 2048 output rows per core (i)
NJB = N // 128              # 32 j-blocks
NTI = NI // 128             # 16 output row-slices (attn@v passes)
SHIFT = 32.0                # softmax shift (cancels in normalization)

# exp-group geometry: et tiles [128, 3, 512] (FD=1536), 4 energy i-chunks
ETG = [(3 * g, min(3, NJB - 3 * g)) for g in range((NJB + 2) // 3)]
NG = len(ETG)               # 11 groups per energy i-chunk
NEIC = NI // 512            # 4 energy i-chunks

DVE_G = (4, 8)              # groups per e-chunk exp'd on DVE (Schraudolph)
CCORR = 8.0
C1 = 128.0 / float(np.log(2.0))
C2 = 127.0 * 128.0 - CCORR
QUOTA = 16                  # attn@v matmuls emitted per pipeline step

_NC_CACHE = {}


def _build_nc():
    from contextlib import ExitStack

    import concourse.bacc as bacc
    import concourse.mybir as mybir
    import concourse.tile as tile

    dt = mybir.dt
    nc = bacc.Bacc(target_bir_lowering=False)

    qt_d = nc.declare_dram_parameter("qt", [128, N], dt.float16, isOutput=False)
    kt_d = nc.declare_dram_parameter("kt", [128, NI], dt.float16, isOutput=False)
    v_d = nc.declare_dram_parameter("v", [8, 128, 4, 132], dt.bfloat16, isOutput=False)
    xres_d = nc.declare_dram_parameter("xres", [NTI, 128, 128], dt.float32, isOutput=False)
    out_d = nc.declare_dram_parameter("out", [NTI, 128, 128], dt.float32, isOutput=True)

    with tile.TileContext(nc) as tc, ExitStack() as ctx:
        persist = ctx.enter_context(tc.tile_pool(name="persist", bufs=1))

        # warm the exp table while DMAs run
        dummy = persist.tile([1, 1], dt.float32)
        nc.vector.memset(dummy[:], 0.0)
        nc.scalar.activation(
            out=dummy[:], in_=dummy[:], func=mybir.ActivationFunctionType.Exp
        )
        warm = persist.tile([128, 64], dt.float16)
        nc.vector.memset(warm[:], 0.0)

        # tiny queue warmers
        qw = persist.tile([1, 4], dt.float16)
        nc.sync.dma_start(out=qw[0:1, 0:2], in_=qt_d[0:1, 0:2])
        nc.gpsimd.dma_start(out=qw[0:1, 2:4], in_=qt_d[0:1, 2:4])

        qt = persist.tile([128, N], dt.float16)
        kt = persist.tile([128, NI], dt.float16)
        v_ch = [
            persist.tile([128, 4, 132], dt.bfloat16, name=f"v{jc}") for jc in range(8)
        ]

        # DMA order mirrors the critical path: the first energy group needs
        # kt chunk 0 + qt cols 0:512; scalar/vector/tensor queues carry no DMA.
        nc.sync.dma_start(out=kt[:, 0:512], in_=kt_d[:, 0:512])
        nc.sync.dma_start(out=qt[:, 0:512], in_=qt_d[:, 0:512])
        nc.gpsimd.dma_start(out=qt[:, 512:1024], in_=qt_d[:, 512:1024])
        nc.gpsimd.dma_start(out=v_ch[0][:], in_=v_d[0])
        nc.sync.dma_start(out=qt[:, 1024:1536], in_=qt_d[:, 1024:1536])
        nc.gpsimd.dma_start(out=v_ch[1][:], in_=v_d[1])
        nc.sync.dma_start(out=kt[:, 512:1024], in_=kt_d[:, 512:1024])
        nc.gpsimd.dma_start(out=qt[:, 1536:2048], in_=qt_d[:, 1536:2048])
        nc.sync.dma_start(out=v_ch[2][:], in_=v_d[2])
        nc.gpsimd.dma_start(out=qt[:, 2048:2560], in_=qt_d[:, 2048:2560])
        nc.sync.dma_start(out=v_ch[3][:], in_=v_d[3])
        nc.gpsimd.dma_start(out=kt[:, 1024:1536], in_=kt_d[:, 1024:1536])
        nc.sync.dma_start(out=qt[:, 2560:3072], in_=qt_d[:, 2560:3072])
        nc.gpsimd.dma_start(out=v_ch[4][:], in_=v_d[4])
        nc.sync.dma_start(out=qt[:, 3072:3584], in_=qt_d[:, 3072:3584])
        nc.gpsimd.dma_start(out=v_ch[5][:], in_=v_d[5])
        nc.sync.dma_start(out=qt[:, 3584:4096], in_=qt_d[:, 3584:4096])
        nc.gpsimd.dma_start(out=kt[:, 1536:2048], in_=kt_d[:, 1536:2048])
        nc.sync.dma_start(out=v_ch[6][:], in_=v_d[6])
        nc.gpsimd.dma_start(out=v_ch[7][:], in_=v_d[7])

        epool = ctx.enter_context(tc.tile_pool(name="ep", bufs=2, space="PSUM"))
        opool = ctx.enter_context(tc.tile_pool(name="op", bufs=1, space="PSUM"))
        ptpool = ctx.enter_context(tc.tile_pool(name="ptp", bufs=24))
        spool = ctx.enter_context(tc.tile_pool(name="sp", bufs=8))
        otpool = ctx.enter_context(tc.tile_pool(name="otp", bufs=4))
        xrpool = ctx.enter_context(tc.tile_pool(name="xrp", bufs=4))

        # PE warmup into the oaA psum slot (released before first real use)
        wt = opool.tile([128, 129], dt.float32, tag="oaA", name="warmp")
        for r in range(8):
            nc.tensor.matmul(wt[0:64, 0:64], warm[:], warm[:], start=True, stop=True)

        flat = [(k, g) for k in range(NEIC) for g in range(NG)]
        ets, pts = {}, {}

        def emit_energy(eic, g):
            jb0, gsz = ETG[g]
            et = epool.tile([128, 3, 512], dt.float32, tag="et", name=f"et{eic}_{g}")
            for r in range(gsz):
                jb = jb0 + r
                for c4 in range(4):
                    nc.tensor.matmul(
                        et[32 * c4 : 32 * c4 + 32, r, :],
                        qt[
                            32 * r : 32 * r + 32,
                            jb * 128 + 32 * c4 : jb * 128 + 32 * c4 + 32,
                        ],
                        kt[32 * r : 32 * r + 32, eic * 512 : (eic + 1) * 512],
                        start=True,
                        stop=True,
                        tile_position=(32 * r, 32 * c4),
                    )
            ets[(eic, g)] = et

        def emit_exp(eic, g):
            gsz = ETG[g][1]
            et = ets.pop((eic, g))
            pt = ptpool.tile([128, 3, 512], dt.uint16, tag="pt", name=f"pt{eic}_{g}")
            if g in DVE_G:
                nc.vector.tensor_scalar(
                    out=pt[:, :gsz, :],
                    in0=et[:, :gsz, :],
                    scalar1=float(C1),
                    scalar2=float(C2),
                    op0=mybir.AluOpType.mult,
                    op1=mybir.AluOpType.add,
                )
            else:
                nc.scalar.activation(
                    out=pt.bitcast(dt.bfloat16)[:, :gsz, :],
                    in_=et[:, :gsz, :],
                    func=mybir.ActivationFunctionType.Exp,
                )
            pts[(eic, g)] = pt

        # --- attn@v pass state (pass p = output rows [128p, 128p+128)) ---
        oa_t, xr_t = {}, {}
        jb_done = [0] * NTI          # next jb to emit for pass p
        blended = [False] * NTI

        def start_pass(p):
            oa_t[p] = opool.tile(
                [128, 129], dt.float32, tag=("oaA" if p % 2 == 0 else "oaB"),
                name=f"oa{p}",
            )
            xr = xrpool.tile([128, 128], dt.float32, tag="xr", name=f"xr{p}")
            nc.gpsimd.dma_start(out=xr[:], in_=xres_d[p])
            xr_t[p] = xr

        def emit_attnv(p, jb):
            eic = p // 4
            ioff = (p % 4) * 128
            g, gg = jb // 3, jb % 3
            pt_bf = pts[(eic, g)].bitcast(mybir.dt.bfloat16)
            nc.tensor.matmul(
                oa_t[p][:],
                pt_bf[:, gg, ioff : ioff + 128],
                v_ch[jb // 4][:, jb % 4, 0:129],
                start=(jb == 0),
                stop=(jb == NJB - 1),
            )

        def emit_blend(p):
            oa = oa_t.pop(p)
            xr = xr_t.pop(p)
            rs = spool.tile([128, 1], dt.float32, tag="rs", name=f"rs{p}")
            nc.vector.reciprocal(rs[:], oa[:, 128:129])
            ot = otpool.tile([128, 128], dt.float32, tag="ot", name=f"ot{p}")
            nc.vector.tensor_scalar(
                out=ot[:],
                in0=oa[:, 0:128],
                scalar1=rs[:],
                scalar2=None,
                op0=mybir.AluOpType.mult,
            )
            nc.gpsimd.tensor_tensor(
                out=ot[:], in0=ot[:], in1=xr[:], op=mybir.AluOpType.add
            )
            nc.sync.dma_start(out=out_d[p], in_=ot[:])
            blended[p] = True

        def drain_attnv(fk, quota):
            """Greedy: emit ready attn@v MMs, lowest pass first.  A pass may
            only exist once its bank-mate (p-2) is blended; group g of chunk k
            is ready strictly after its exp step (PE-FIFO safety)."""
            for p in range(NTI):
                if quota <= 0:
                    return
                if blended[p]:
                    continue
                if p >= 2 and not blended[p - 2]:
                    return
                # groups of chunk p//4 with flat index < fk are safe to consume
                gready = min(NG, fk - (p // 4) * NG)
                if gready <= 0:
                    return
                jb_ready = ETG[gready - 1][0] + ETG[gready - 1][1]
                if p not in oa_t and jb_done[p] < jb_ready:
                    start_pass(p)
                while jb_done[p] < jb_ready and quota > 0:
                    emit_attnv(p, jb_done[p])
                    jb_done[p] += 1
                    quota -= 1
                if jb_done[p] == NJB:
                    emit_blend(p)

        emit_energy(0, 0)
        emit_energy(0, 1)
        for fk, (eic, g) in enumerate(flat):
            for ahead in (1, 2):
                if fk + ahead < len(flat) and flat[fk + ahead] not in ets:
                    emit_energy(*flat[fk + ahead])
            emit_exp(eic, g)
            drain_attnv(fk, QUOTA)
        while not all(blended):
            drain_attnv(len(flat), QUOTA)

    nc.finalize()
    return nc


def get_nc():
    if "nc" not in _NC_CACHE:
        _NC_CACHE["nc"] = _build_nc()
    return _NC_CACHE["nc"]


def _to_bf16(a):
    import ml_dtypes

    return a.astype(ml_dtypes.bfloat16)


def make_in_maps(x, Wq, Wk, Wv, gamma):
    x = np.asarray(x, dtype=np.float64)
    Wq = np.asarray(Wq, dtype=np.float64)
    Wk = np.asarray(Wk, dtype=np.float64)
    Wv = np.asarray(Wv, dtype=np.float64)
    gamma = float(np.asarray(gamma).reshape(-1)[0])

    xf = x.reshape(B, N, C)
    in_maps = []
    for c in range(NCORES):
        b, ih = c // 2, c % 2
        q = xf[b] @ Wq                      # [N, 16]
        k = xf[b] @ Wk                      # [N, 16]
        v = gamma * (xf[b] @ Wv)            # [N, C], gamma folded in
        isl = slice(ih * NI, (ih + 1) * NI)

        qt = np.zeros((128, N), dtype=np.float16)
        ktr = np.zeros((128, NI), dtype=np.float16)
        for r in range(4):
            qt[32 * r : 32 * r + 16] = q.T.astype(np.float16)
            qt[32 * r + 16] = 1.0
            ktr[32 * r : 32 * r + 16] = k[isl].T.astype(np.float16)
            ktr[32 * r + 16] = -SHIFT
        vv = np.zeros((8, 128, 4, 132), dtype=np.float32)
        vr = v.reshape(8, 4, 128, C)
        for jc in range(8):
            for kk in range(4):
                vv[jc, :, kk, 0:128] = vr[jc, kk]
                vv[jc, :, kk, 128] = 1.0
        in_maps.append(
            {
                "qt": np.ascontiguousarray(qt),
                "kt": np.ascontiguousarray(ktr),
                "v": _to_bf16(vv),
                "xres": np.ascontiguousarray(
                    xf[b][isl].reshape(NTI, 128, 128).astype(np.float32)
                ),
            }
        )
    return in_maps


def assemble_out(results):
    outs = [np.asarray(results[c]["out"]).reshape(NI, C) for c in range(NCORES)]
    full = np.stack(
        [np.concatenate([outs[2 * b], outs[2 * b + 1]], axis=0) for b in range(B)]
    )
    return full.reshape(B, Dd, Hh, Ww, C).astype(np.float32)


def kernel(x, Wq, Wk, Wv, gamma):
    from concourse.bass_utils import run_bass_kernel_spmd

    nc = get_nc()
    in_maps = make_in_maps(x, Wq, Wk, Wv, gamma)
    res = run_bass_kernel_spmd(nc, in_maps, core_ids=list(range(NCORES)))
    return assemble_out(res.results)
